# revision 1
# baseline (speedup 1.0000x reference)
"""AttentionBlock3D (GroupNorm + 8-head self-attention + out-proj + residual)
as a Trainium2 Bass/Tile SPMD kernel over 8 NeuronCores.

Sharding: token-parallel. Core i handles batch b = i//4 and a 1024-token
slice j = i%4 of the 4096 tokens. Each core receives its batch's x
*rolled* so that its token slice sits at columns 0:1024 — attention is
permutation-invariant in the key/value axis, so one SPMD program serves
all cores with no core-id-dependent slicing and no collectives:

  - GroupNorm over the full batch (stats are slice-invariant)
  - K, V projections for all 4096 (rolled) tokens, Q only for cols 0:1024
  - scores^T = K^T-tiles x Q (t on partitions), softmax via exp on ACT
    (no max subtraction needed: |scores| <~ 8 so fp32 exp is safe),
    denominator via a ones-column appended to V^T in the PV matmul
  - out-proj over the full 256 channels locally, + bias + residual

All matmuls run as float32r (full-rate fp32 mode for free dims >= 256).

Sync-wait discipline: this compiler allows only ONE semaphore wait on
fused 4-byte matmuls and ACT instructions. The kernel is arranged so
every PE/ACT instruction depends on at most one foreign engine:
  - all DMA'd operands are staged through a DVE copy before compute
  - group masks / ones columns are built by DVE memsets, not DMA
  - softmax normalization reads PSUM only from DVE, broadcast on GPSIMD
  - a spare-slot ST matmul pins the DVE->PE dep that retires the PV
    accumulator bank (so the next PV start=True matmul needs no DVE wait)
"""

import os

import numpy as np

import concourse.bass as bass
import concourse.tile as tile
from concourse import mybir
from concourse.bass_utils import run_bass_kernel_spmd
from concourse.tile import add_dep_helper


def _install_ntff_hook():
    """Provide antenv.axon_hooks if the image lacks it, so that
    run_bass_kernel_spmd(trace=True) can capture NTFF profiles under axon."""
    import sys as _sys
    import types as _types
    name = "antenv.axon_hooks"
    if name in _sys.modules:
        return
    try:
        import importlib
        importlib.import_module(name)
        return
    except ImportError:
        pass
    hook = None
    try:
        from trn_agent_boot.trn_boot import _ntff_profile_via_ctypes
        hook = _ntff_profile_via_ctypes("/opt/axon/libaxon_pjrt.so")
    except Exception:
        hook = None
    mod = _types.ModuleType(name)
    mod.get_axon_ntff_profile_hook = lambda: hook
    _sys.modules[name] = mod


_install_ntff_hook()

F32 = mybir.dt.float32
F32R = mybir.dt.float32r
BF16 = mybir.dt.bfloat16
AF = mybir.ActivationFunctionType
ALU = mybir.AluOpType

B, C, N = 2, 256, 4096          # batch, channels, tokens (16*16*16)
H, D, G = 8, 32, 8              # heads, head dim, groupnorm groups
S = 1024                        # tokens per core
NCORES = 8
EPS = 1e-5
SCALE = float(D) ** -0.5
CT = 2                          # channel tiles of 128
TT = N // 128                   # 32 t-tiles of 128 tokens
SB = S // 512                   # 2 s-blocks of 512
QUAD = 3                        # t-tiles per exp batch (3 PSUM banks)


def _strip_same_engine_waits(nc):
    """Drop semaphore waits an instruction holds on its own engine.

    PE, ACT and DVE execute strict-FIFO, so program order already orders
    same-engine dependencies. Tile still emits self-waits for buffer-reuse
    WAW edges, and those overflow this compiler's one-sync-wait ISA budget
    on fused matmul / activation instructions."""
    # Only PE and ACT: their fused-instruction ISA structs hold a single
    # sync wait, and both engines are pc-monotone so dropping self-waits is
    # sound. DVE keeps its self-waits — its write pipeline is deep enough
    # that back-to-back RAW without a sem has been observed to race on HW.
    own = {mybir.EngineType.PE: "PE_",
           mybir.EngineType.Activation: "Activation_"}
    for fn in nc.m.functions:
        for blk in fn.blocks:
            for inst in blk.instructions:
                pref = own.get(inst.engine)
                if pref is None:
                    continue
                si = inst.sync_info
                if si is None or not si.on_wait:
                    continue
                kept = [w for w in si.on_wait
                        if not (w.sync_type == "semaphore"
                                and (w.ant_name or "").startswith(pref))]
                if len(kept) != len(si.on_wait):
                    si.on_wait = kept
                    inst.sync_info = si

def _fix_dram_readback_waits(nc):
    """Tile sometimes elides the wait on the second+ DMA reading a DRAM
    scratch tile back, assuming transitive coverage that does not hold
    across DMA queues. Force every DRAM read-back to wait on the queue
    sem value of the store that produced its data."""
    import bass_rust as _br
    dram_names = set()
    for fn in nc.m.functions:
        for alloc in fn.allocations:
            for ml in getattr(alloc, "memorylocations", None) or []:
                if str(getattr(ml, "type", "")) == "DRAM":
                    dram_names.add(ml.name)
    def _rng(ap_obj):
        off = getattr(ap_obj, "offset", 0) or 0
        n = 1
        for pair in list(getattr(ap_obj, "ap", []) or []):
            n *= pair[1]
        return (off, off + n)

    sem_cum = {}
    stores = {}  # memref -> list of (start, end, ant, val, sid)
    for fn in nc.m.functions:
        for blk in fn.blocks:
            for inst in blk.instructions:
                si = inst.sync_info
                ups = list(si.on_update) if si else []
                if "DMACopy" in type(inst).__name__ and si is not None:
                    new_waits = list(si.on_wait)
                    for x in inst.ins:
                        n = getattr(x, "memref", None)
                        if n not in dram_names or n not in stores:
                            continue
                        lo, hi = _rng(x)
                        for s0, s1, ant, val, sid in stores[n]:
                            if s1 <= lo or s0 >= hi:
                                continue
                            if not any(w.ant_name == ant
                                       and w.wait_value >= val
                                       for w in new_waits):
                                new_waits.append(_br.SyncWait(
                                    sync_type="semaphore", id=sid,
                                    ant_name=ant,
                                    wait_mode="sem-ge-imm",
                                    wait_value=val, wait_reg=None))
                    if len(new_waits) != len(si.on_wait):
                        si.on_wait = new_waits
                        inst.sync_info = si
                for u in ups:
                    if (u.ant_name or "").startswith(("DMASW", "DMAHW")):
                        sem_cum[u.ant_name] = sem_cum.get(u.ant_name, 0) \
                            + u.update_value
                if "DMACopy" in type(inst).__name__:
                    for o in inst.outs:
                        n = getattr(o, "memref", None)
                        if n in dram_names:
                            lo, hi = _rng(o)
                            for u in ups:
                                if (u.ant_name or "").startswith(
                                        ("DMASW", "DMAHW")):
                                    stores.setdefault(n, []).append(
                                        (lo, hi, u.ant_name,
                                         sem_cum[u.ant_name], u.id))


def _prune_tail_drain(nc):
    """The kernel-tail drain waits on every engine and every DMA queue used,
    which overflows the drain ISA struct's wait slots. Every input DMA here
    is awaited by a compute consumer and every compute chain feeds the
    output stores, so the only waits the drain truly needs are the compute
    engines and the queue sems of DMAs that write ExternalOutput tensors."""
    out_names = set()
    for fn in nc.m.functions:
        for alloc in fn.allocations:
            if getattr(alloc, "kind", None) == "ExternalOutput":
                for ml in alloc.memorylocations:
                    out_names.add(ml.name)
    keep_dma_sems = set()
    for fn in nc.m.functions:
        for blk in fn.blocks:
            for inst in blk.instructions:
                if "DMACopy" not in type(inst).__name__:
                    continue
                if any(getattr(o, "memref", None) in out_names
                       for o in inst.outs):
                    si = inst.sync_info
                    if si:
                        for u in si.on_update:
                            keep_dma_sems.add(u.ant_name)
    for fn in nc.m.functions:
        for blk in fn.blocks:
            for inst in blk.instructions:
                if "Drain" not in type(inst).__name__:
                    continue
                si = inst.sync_info
                if si is None or len(si.on_wait) <= 2:
                    continue
                kept = [w for w in si.on_wait
                        if w.ant_name in keep_dma_sems]
                if len(kept) != len(si.on_wait):
                    si.on_wait = kept
                    inst.sync_info = si


def build_program():
    nc = bass.Bass()

    xb = nc.dram_tensor("xb", [C, N], F32R, kind="ExternalInput")
    wqk = nc.dram_tensor("wqk", [C, 2 * C], F32R, kind="ExternalInput")
    wv = nc.dram_tensor("wv", [C, H * (D + 1)], F32R, kind="ExternalInput")
    wo = nc.dram_tensor("wo", [C, C], F32R, kind="ExternalInput")
    bo = nc.dram_tensor("bo", [C, 1], F32, kind="ExternalInput")
    nw = nc.dram_tensor("nw", [C, 1], F32, kind="ExternalInput")
    nb = nc.dram_tensor("nb", [C, 1], F32, kind="ExternalInput")
    out = nc.dram_tensor("out", [C, S], F32, kind="ExternalOutput")

    with tile.TileContext(nc) as tc:
        with (
            tc.tile_pool(name="const", bufs=1) as cpool,
            tc.tile_pool(name="wst", bufs=1) as wst,
            tc.tile_pool(name="xpool", bufs=1) as xpool,
            tc.tile_pool(name="kq", bufs=1) as kqpool,
            tc.tile_pool(name="vt", bufs=1) as vtpool,
            tc.tile_pool(name="small", bufs=1) as small,
            tc.tile_pool(name="dram", bufs=1, space="DRAM") as dpool,
            tc.tile_pool(name="psum", bufs=2, space="PSUM") as psum,
        ):
            # ---- group masks for the stat matmuls, via DVE memsets ----
            gmask = [cpool.tile([128, G], F32, name=f"gmask{i}", tag=f"gmask{i}")
                     for i in range(CT)]
            for ct in range(CT):
                for quad in range(4):
                    rows = slice(32 * quad, 32 * (quad + 1))
                    other = slice(4 - 4 * ct, 8 - 4 * ct)
                    nc.vector.memset(gmask[ct][rows, other], 0.0)
                    for g in range(4):
                        col = slice(4 * ct + g, 4 * ct + g + 1)
                        nc.vector.memset(gmask[ct][rows, col],
                                         1.0 if g == quad else 0.0)
            zerob = cpool.tile([128, 1], F32, name="zerob", tag="zerob")
            nc.vector.memset(zerob[:], 0.0)
            epsb = cpool.tile([G, 1], F32, name="epsb", tag="epsb")
            nc.vector.memset(epsb[:], EPS)
            ones_f = wst.tile([128, G], F32, name="ones_f", tag="ones_f")
            nc.vector.memset(ones_f[:], 1.0)
            ones8r = cpool.tile([128, G], BF16, name="ones8r", tag="ones8r")
            nc.vector.tensor_copy(ones8r[:], ones_f[:])
            # warm the natural_log_exp_and_others ACT table set with a dummy
            # op whose only dependency is DVE, so no later activation pays
            # the table-load pseudo-instruction's extra sync wait
            actwarm = cpool.tile([128, 1], F32, name="actwarm", tag="actwarm")
            nc.scalar.activation(actwarm[:], zerob[:], AF.Ln, bias=1.0,
                                 scale=1.0)

            # ---- weights: DMA into staging, DVE-copy to compute tiles so
            # downstream PE/ACT instructions never wait on DMA queues ----
            wqk_s = [wst.tile([128, 2 * C], F32R, name=f"wqks{i}", tag=f"wqks{i}")
                     for i in range(CT)]
            wv_s = [wst.tile([128, H * (D + 1)], F32R, name=f"wvs{i}", tag=f"wvs{i}")
                    for i in range(CT)]
            wo_s = [wst.tile([128, C], F32R, name=f"wos{i}", tag=f"wos{i}")
                    for i in range(CT)]
            nwnb_s = [wst.tile([128, 3], F32, name=f"nns{i}", tag=f"nns{i}")
                      for i in range(CT)]
            wqk_t = [cpool.tile([128, 2 * C], F32R, name=f"wqk{i}", tag=f"wqk{i}")
                     for i in range(CT)]
            wv_t = [cpool.tile([128, H * (D + 1)], F32R, name=f"wv{i}", tag=f"wv{i}")
                    for i in range(CT)]
            wo_t = [cpool.tile([128, C], F32R, name=f"wo{i}", tag=f"wo{i}")
                    for i in range(CT)]
            nw_t = [cpool.tile([128, 1], F32, name=f"nw{i}", tag=f"nw{i}")
                    for i in range(CT)]
            nb_t = [cpool.tile([128, 1], F32, name=f"nb{i}", tag=f"nb{i}")
                    for i in range(CT)]
            bo_t = [cpool.tile([128, 1], F32, name=f"bo{i}", tag=f"bo{i}")
                    for i in range(CT)]
            x_t = [xpool.tile([128, N], F32R, name=f"x{i}", tag=f"x{i}")
                   for i in range(CT)]
            for ct in range(CT):
                nc.sync.dma_start(x_t[ct][:], xb[128 * ct:128 * (ct + 1), :])
            for ct in range(CT):
                sl = slice(128 * ct, 128 * (ct + 1))
                nc.sync.dma_start(wqk_s[ct][:], wqk[sl, :])
                nc.sync.dma_start(wv_s[ct][:], wv[sl, :])
                nc.sync.dma_start(wo_s[ct][:], wo[sl, :])
                nc.sync.dma_start(nwnb_s[ct][:, 0:1], nw[sl, :])
                nc.sync.dma_start(nwnb_s[ct][:, 1:2], nb[sl, :])
                nc.sync.dma_start(nwnb_s[ct][:, 2:3], bo[sl, :])
            for ct in range(CT):
                nc.vector.tensor_copy(wqk_t[ct][:], wqk_s[ct][:])
                nc.vector.tensor_copy(wv_t[ct][:], wv_s[ct][:])
                nc.vector.tensor_copy(wo_t[ct][:], wo_s[ct][:])
                nc.vector.tensor_copy(nw_t[ct][:], nwnb_s[ct][:, 0:1])
                nc.vector.tensor_copy(nb_t[ct][:], nwnb_s[ct][:, 1:2])
                nc.vector.tensor_copy(bo_t[ct][:], nwnb_s[ct][:, 2:3])

            # ---- load x ----
            xres = [xpool.tile([128, S], F32, name=f"xres{i}", tag=f"xres{i}")
                    for i in range(CT)]
            for ct in range(CT):
                nc.vector.tensor_copy(xres[ct][:], x_t[ct][:, 0:S])

            attn_d = [kqpool.tile([128, S], F32R, name=f"at{i}", tag=f"at{i}")
                      for i in range(CT)]
            k_t = [kqpool.tile([128, N], BF16, name=f"k{i}", tag=f"k{i}")
                   for i in range(CT)]
            q_t = [kqpool.tile([128, S], BF16, name=f"q{i}", tag=f"q{i}")
                   for i in range(CT)]
            vt_all = vtpool.tile([128, TT, H, D + 1], BF16, name="vt", tag="vt")
            # persistent, manually double-buffered exp output (a rotating
            # pool slot here would add a second sync wait on the ACT op)
            e_all = vtpool.tile([128, 2, QUAD, 512], BF16, name="e_all",
                                tag="e_all")

            # ---- groupnorm stats ----
            stats = [small.tile([128, 2], F32, name=f"st{i}", tag=f"st{i}")
                     for i in range(CT)]
            xsq = [wst.tile([128, N], F32, name=f"xsq{i}", tag=f"xsq{i}")
                   for i in range(CT)]
            if True:
                gstat_p = psum.tile([G, 2], F32, name="gstat", tag="pv")
                for ct in range(CT):
                    nc.vector.reduce_sum(stats[ct][:, 0:1], x_t[ct][:],
                                         axis=mybir.AxisListType.X)
                    # x*x into a scratch tile, then reduce
                    nc.vector.tensor_mul(xsq[ct][:], x_t[ct][:], x_t[ct][:])
                    nc.vector.reduce_sum(stats[ct][:, 1:2], xsq[ct][:],
                                         axis=mybir.AxisListType.X)
                for ct in range(CT):
                    nc.tensor.matmul(gstat_p[:], gmask[ct][:], stats[ct][:],
                                     start=(ct == 0), stop=(ct == CT - 1))
                MTOT = float(32 * N)
                ms = small.tile([G, 2], F32, name="ms", tag="ms")   # [mean, rstd]
                ex2 = small.tile([G, 1], F32, name="ex2", tag="ex2")
                nc.vector.tensor_scalar_mul(ms[:, 0:1], gstat_p[:, 0:1], 1.0 / MTOT)
                nc.vector.tensor_scalar_mul(ex2[:], gstat_p[:, 1:2], 1.0 / MTOT)
            m2 = small.tile([G, 1], F32, name="m2", tag="m2")
            nc.vector.tensor_mul(m2[:], ms[:, 0:1], ms[:, 0:1])
            var = small.tile([G, 1], F32, name="var", tag="var")
            nc.vector.tensor_sub(var[:], ex2[:], m2[:])
            sd = small.tile([G, 1], F32, name="sd", tag="sd")
            nc.scalar.activation(sd[:], var[:], AF.Ln, bias=epsb[:], scale=1.0)
            rstd8 = small.tile([G, 1], F32, name="rstd8", tag="rstd8")
            nc.scalar.activation(rstd8[:], sd[:], AF.Exp, bias=zerob[0:G, :],
                                 scale=-0.5)
            nc.vector.tensor_copy(ms[:, 1:2], rstd8[:])

            # broadcast [8,2] group stats to [128,2] per c-tile via a DRAM
            # round-trip with a replicating read pattern
            ms_d = dpool.tile([G, 2], F32, name="msd", tag="msd")
            nc.gpsimd.dma_start(ms_d[:], ms[:])
            mr = [small.tile([128, 2], F32, name=f"mr{i}", tag=f"mr{i}")
                  for i in range(CT)]
            for ct in range(CT):
                src = bass.AP(tensor=ms_d.tensor,
                              offset=ms_d.offset + 8 * ct,
                              ap=[[2, 4], [0, 32], [1, 2]])
                nc.gpsimd.dma_start(mr[ct][:], src)

            # per-channel affine: xn = x * weff + beff (in place on x_t)
            for ct in range(CT):
                weff = small.tile([128, 1], F32, name=f"weff{ct}", tag=f"weff{ct}")
                beff = small.tile([128, 1], F32, name=f"beff{ct}", tag=f"beff{ct}")
                nc.vector.tensor_mul(weff[:], mr[ct][:, 1:2], nw_t[ct][:])
                nc.vector.tensor_mul(beff[:], mr[ct][:, 0:1], weff[:])
                nc.vector.tensor_sub(beff[:], nb_t[ct][:], beff[:])
                nc.vector.tensor_scalar(out=x_t[ct][:], in0=x_t[ct][:],
                                        scalar1=weff[:], scalar2=beff[:],
                                        op0=ALU.mult, op1=ALU.add)
            xn = x_t  # normalized in place

            # ---- projections: V^T first (so its DVE writes are observed by
            # the PE before any attention matmul), then Q, then K ----
            if True:
                # V^T: [token, head*(D+1)] directly from xn as stationary
                for ti in range(TT):
                    vp = psum.tile([128, H * (D + 1)], F32, name="vp", tag="sc")
                    for ci in range(CT):
                        nc.tensor.matmul(
                            vp[:],
                            xn[ci][:, 128 * ti:128 * (ti + 1)],
                            wv_t[ci][:],
                            start=(ci == 0), stop=(ci == CT - 1))
                    nc.vector.tensor_copy(vt_all[:, ti, :, 0:D],
                                          vp[:].rearrange(
                                              "p (h e) -> p h e",
                                              e=D + 1)[:, :, 0:D])
                    # each head's extra column holds ones (the PV
                    # denominator row); strided copy from the ones tile
                    nc.vector.tensor_copy(vt_all[:, ti, :, D], ones8r[:])
                # Q: out channels 0:256 of wqk cols; only S token columns
                for sblk in range(SB):
                    for m in range(CT):
                        qp = psum.tile([128, 512], F32, name="qp", tag="sc")
                        for ci in range(CT):
                            nc.tensor.matmul(
                                qp[:],
                                wqk_t[ci][:, 128 * m:128 * (m + 1)],
                                xn[ci][:, 512 * sblk:512 * (sblk + 1)],
                                start=(ci == 0), stop=(ci == CT - 1))
                        nc.vector.tensor_copy(
                            q_t[m][:, 512 * sblk:512 * (sblk + 1)], qp[:])
                # K: out channels 256:512 of wqk cols; all N tokens
                for nblk in range(N // 512):
                    for m in range(CT):
                        kp = psum.tile([128, 512], F32, name="kp", tag="sc")
                        for ci in range(CT):
                            nc.tensor.matmul(
                                kp[:],
                                wqk_t[ci][:, C + 128 * m:C + 128 * (m + 1)],
                                xn[ci][:, 512 * nblk:512 * (nblk + 1)],
                                start=(ci == 0), stop=(ci == CT - 1))
                        nc.vector.tensor_copy(
                            k_t[m][:, 512 * nblk:512 * (nblk + 1)], kp[:])

            # ---- attention ----
            attn = attn_d
            rcp_all = small.tile([1, 512 * 2 * H], F32, name="rcp_all",
                                 tag="rcp_all")
            if True:
                muls = []  # normalize/recip per group, for pv-bank retire
                for h in range(H):
                    htile, hp = h // 4, 32 * (h % 4)
                    for sblk in range(SB):
                        grp = 2 * h + sblk
                        ssl = slice(512 * sblk, 512 * (sblk + 1))
                        pv = psum.tile([33, 512], F32, name="pv", tag="pv")
                        nq = (TT + QUAD - 1) // QUAD
                        for qd in range(nq):
                            t0 = QUAD * qd
                            nt = min(QUAD, TT - t0)
                            st = psum.tile([128, QUAD, 512], F32, name="st",
                                           tag="sc")
                            last_st = None
                            for j in range(nt):
                                ti = t0 + j
                                last_st = nc.tensor.matmul(
                                    st[:, j, :],
                                    k_t[htile][hp:hp + 32,
                                               128 * ti:128 * (ti + 1)],
                                    q_t[htile][hp:hp + 32, ssl],
                                    start=True, stop=True,
                                    tile_position=(hp, 0))
                            if qd == 0 and grp >= 2:
                                # pin the DVE->PE edge that retires the PV
                                # bank two groups back on a matmul with a
                                # spare wait slot; the next PV start=True
                                # then needs no DVE wait of its own
                                add_dep_helper(last_st.ins, muls[grp - 2].ins,
                                               reason="retire pv bank")
                            e = e_all[:, qd % 2]
                            nc.scalar.activation(e[:, 0:nt, :], st[:, 0:nt, :],
                                                 AF.Exp, bias=zerob[:],
                                                 scale=SCALE)
                            for j in range(nt):
                                ti = t0 + j
                                nc.tensor.matmul(
                                    pv[:],
                                    vt_all[:, ti, h, :],
                                    e[:, j, :],
                                    start=(ti == 0), stop=(ti == TT - 1))
                        # unnormalized O to SBUF; reciprocal of the
                        # denominator row into the per-group collector
                        # (normalization is applied in bulk afterwards)
                        nc.vector.tensor_copy(attn[htile][hp:hp + 32, ssl],
                                              pv[0:D, :])
                        mul = nc.vector.reciprocal(
                            rcp_all[:, 512 * grp:512 * (grp + 1)],
                            pv[D:D + 1, :])
                        muls.append(mul)

            # ---- bulk softmax normalization: broadcast the collected
            # 1/denom values to [128, S] per c-tile via a DRAM round-trip,
            # then scale the unnormalized attention outputs in place ----
            rcp_d = dpool.tile([512 * 2 * H], F32, name="rcpd", tag="rcpd")
            nc.gpsimd.dma_start(rcp_d[:], rcp_all[:])
            for ci in range(CT):
                rbc = small.tile([128, S], F32, name=f"rbc{ci}", tag=f"rbc{ci}")
                bsrc = bass.AP(tensor=rcp_d.tensor,
                               offset=rcp_d.offset + 4096 * ci,
                               ap=[[1024, 4], [0, 32], [512, 2], [1, 512]])
                nc.gpsimd.dma_start(rbc[:], bsrc)
                nc.vector.tensor_mul(attn[ci][:], attn[ci][:], rbc[:])

            # ---- output projection + bias + residual ----
            out_all = small.tile([128, CT * S], F32, name="out_all",
                                 tag="out_all")
            out_t = [out_all[:, S * i:S * (i + 1)] for i in range(CT)]
            if True:
                for sblk in range(SB):
                    ssl = slice(512 * sblk, 512 * (sblk + 1))
                    for m in range(CT):
                        op = psum.tile([128, 512], F32, name="op", tag="pv")
                        for ci in range(CT):
                            nc.tensor.matmul(
                                op[:],
                                wo_t[ci][:, 128 * m:128 * (m + 1)],
                                attn[ci][:, ssl],
                                start=(ci == 0), stop=(ci == CT - 1))
                        nc.vector.scalar_tensor_tensor(
                            out=out_t[m][:, ssl], in0=op[:], scalar=bo_t[m][:],
                            in1=xres[m][:, ssl], op0=ALU.add, op1=ALU.add)

            out_r = out.rearrange("(m p) s -> p m s", p=128)
            nc.gpsimd.dma_start(
                out_r, out_all[:].rearrange("p (m s) -> p m s", m=CT))

    _strip_same_engine_waits(nc)
    _fix_dram_readback_waits(nc)
    _prune_tail_drain(nc)
    return nc


def shard_inputs(x, norm_w, norm_b, w_qkv, w_out, b_out):
    """Build the 8 per-core input maps from the full problem inputs."""
    x = np.asarray(x, np.float32).reshape(B, C, N)
    w_qkv = np.asarray(w_qkv, np.float32)
    w_out = np.asarray(w_out, np.float32)
    wqkT = np.ascontiguousarray(w_qkv[:2 * C].T)          # [C, 512]
    wv = np.zeros((C, H * (D + 1)), np.float32)           # [C, 264]
    for h in range(H):
        wv[:, (D + 1) * h:(D + 1) * h + D] = w_qkv[2 * C + D * h:2 * C + D * (h + 1)].T
    woT = np.ascontiguousarray(w_out.T)
    nwc = np.ascontiguousarray(np.asarray(norm_w, np.float32).reshape(C, 1))
    nbc = np.ascontiguousarray(np.asarray(norm_b, np.float32).reshape(C, 1))
    boc = np.ascontiguousarray(np.asarray(b_out, np.float32).reshape(C, 1))
    in_maps = []
    for i in range(NCORES):
        b, j = i // 4, i % 4
        xrot = np.ascontiguousarray(np.roll(x[b], -S * j, axis=1))
        in_maps.append({"xb": xrot, "wqk": wqkT, "wv": wv, "wo": woT,
                        "bo": boc, "nw": nwc, "nb": nbc})
    return in_maps


def kernel_with_info(x, norm_w, norm_b, w_qkv, w_out, b_out):
    in_maps = shard_inputs(x, norm_w, norm_b, w_qkv, w_out, b_out)
    nc = build_program()
    trace = bool(os.environ.get("BASS_TRACE"))
    res = run_bass_kernel_spmd(nc, in_maps, core_ids=list(range(NCORES)),
                               trace=trace)
    full = np.empty((B, C, N), np.float32)
    for i in range(NCORES):
        b, j = i // 4, i % 4
        full[b][:, S * j:S * (j + 1)] = res.results[i]["out"]
    return full.reshape(B, C, 16, 16, 16), res


def kernel(x, norm_w, norm_b, w_qkv, w_out, b_out):
    out, _ = kernel_with_info(x, norm_w, norm_b, w_qkv, w_out, b_out)
    return out



# revision 2
# speedup vs baseline: 1.3909x; 1.3909x over previous
"""AttentionBlock3D (GroupNorm + 8-head self-attention + out-proj + residual)
as a Trainium2 Bass/Tile SPMD kernel over 8 NeuronCores.

Sharding: token-parallel. Core i handles batch b = i//4 and a 1024-token
slice j = i%4 of the 4096 tokens. Each core receives its batch's x
*rolled* so that its token slice sits at columns 0:1024 — attention is
permutation-invariant in the key/value axis, so one SPMD program serves
all cores with no core-id-dependent slicing and no collectives:

  - GroupNorm over the full batch (stats are slice-invariant)
  - K, V projections for all 4096 (rolled) tokens, Q only for cols 0:1024
  - attention processed as a flat software-pipelined stream of
    (head-pair, s-block, t-tile) steps:
      * QK: two row-tiled matmuls (heads at PE row bands 0 and 64) run
        CONCURRENTLY via tile_position — contraction is only D=32, so
        packing two heads recovers 2x tensor throughput
      * exp on ACT over both heads' score banks at once ([128, 1024]
        activations keep the ACT stream dense — ACT is the bottleneck
        engine at ~1 elem/lane/cycle)
      * PV: two col-tiled matmuls (M=D+1=33 at col positions 0 and 64)
        run concurrently; the 33rd stationary column is ones, so the
        accumulator's partition 32/96 rows collect the softmax
        denominators for free
      * the PV(k-1) matmuls are emitted AFTER QK(k) so the PE never
        stalls waiting for exp(k-1) — exp runs in the shadow of QK(k)
  - denominator reciprocals via exp(-ln(x)) on ACT (same table set as
    the softmax exp; the old per-group DVE reciprocal was 8 cyc/elem on
    a single lane), broadcast to [128, S] via a DRAM round-trip
  - out-proj over the full 256 channels locally, + bias + residual

Sync-wait discipline: this compiler allows only ONE semaphore wait on
fused 4-byte matmuls and ACT instructions. The kernel is arranged so
every PE/ACT instruction depends on at most one foreign engine:
  - all DMA'd operands are staged through a DVE copy before compute
  - group masks / ones columns are built by DVE memsets, not DMA
  - exp's RAW (on QK) and WAR (on PV two steps back) are both PE deps
    and merge into one wait; QK's only foreign dep is exp's ACT sem
  - the second QK matmul of a step needs no wait of its own (its WAR is
    covered transitively by the first), so the PV-accumulator retire
    dep (a DVE copyout two groups back) is pinned there
"""

import os

import numpy as np

import concourse.bass as bass
import concourse.tile as tile
from concourse import mybir
from concourse.bass_utils import run_bass_kernel_spmd
from concourse.tile import add_dep_helper


def _install_ntff_hook():
    """Provide antenv.axon_hooks if the image lacks it, so that
    run_bass_kernel_spmd(trace=True) can capture NTFF profiles under axon."""
    import sys as _sys
    import types as _types
    name = "antenv.axon_hooks"
    if name in _sys.modules:
        return
    try:
        import importlib
        importlib.import_module(name)
        return
    except ImportError:
        pass
    hook = None
    try:
        from trn_agent_boot.trn_boot import _ntff_profile_via_ctypes
        hook = _ntff_profile_via_ctypes("/opt/axon/libaxon_pjrt.so")
    except Exception:
        hook = None
    mod = _types.ModuleType(name)
    mod.get_axon_ntff_profile_hook = lambda: hook
    _sys.modules[name] = mod


_install_ntff_hook()

F32 = mybir.dt.float32
F32R = mybir.dt.float32r
BF16 = mybir.dt.bfloat16
AF = mybir.ActivationFunctionType
ALU = mybir.AluOpType

B, C, N = 2, 256, 4096          # batch, channels, tokens (16*16*16)
H, D, G = 8, 32, 8              # heads, head dim, groupnorm groups
S = 1024                        # tokens per core
NCORES = 8
EPS = 1e-5
SCALE = float(D) ** -0.5
CT = 2                          # channel tiles of 128
TT = N // 128                   # 32 t-tiles of 128 tokens
SB = S // 512                   # 2 s-blocks of 512


def _strip_same_engine_waits(nc):
    """Drop semaphore waits an instruction holds on its own engine.

    PE, ACT and DVE execute strict-FIFO, so program order already orders
    same-engine dependencies. Tile still emits self-waits for buffer-reuse
    WAW edges, and those overflow this compiler's one-sync-wait ISA budget
    on fused matmul / activation instructions."""
    # Only PE and ACT: their fused-instruction ISA structs hold a single
    # sync wait, and both engines are pc-monotone so dropping self-waits is
    # sound. DVE keeps its self-waits — its write pipeline is deep enough
    # that back-to-back RAW without a sem has been observed to race on HW.
    own = {mybir.EngineType.PE: "PE_",
           mybir.EngineType.Activation: "Activation_"}
    for fn in nc.m.functions:
        for blk in fn.blocks:
            for inst in blk.instructions:
                pref = own.get(inst.engine)
                if pref is None:
                    continue
                si = inst.sync_info
                if si is None or not si.on_wait:
                    continue
                kept = [w for w in si.on_wait
                        if not (w.sync_type == "semaphore"
                                and (w.ant_name or "").startswith(pref))]
                if len(kept) != len(si.on_wait):
                    si.on_wait = kept
                    inst.sync_info = si

def _fix_dram_readback_waits(nc):
    """Tile sometimes elides the wait on the second+ DMA reading a DRAM
    scratch tile back, assuming transitive coverage that does not hold
    across DMA queues. Force every DRAM read-back to wait on the queue
    sem value of the store that produced its data."""
    import bass_rust as _br
    dram_names = set()
    for fn in nc.m.functions:
        for alloc in fn.allocations:
            for ml in getattr(alloc, "memorylocations", None) or []:
                if str(getattr(ml, "type", "")) == "DRAM":
                    dram_names.add(ml.name)
    def _rng(ap_obj):
        off = getattr(ap_obj, "offset", 0) or 0
        n = 1
        for pair in list(getattr(ap_obj, "ap", []) or []):
            n *= pair[1]
        return (off, off + n)

    sem_cum = {}
    stores = {}  # memref -> list of (start, end, ant, val, sid)
    for fn in nc.m.functions:
        for blk in fn.blocks:
            for inst in blk.instructions:
                si = inst.sync_info
                ups = list(si.on_update) if si else []
                if "DMACopy" in type(inst).__name__ and si is not None:
                    new_waits = list(si.on_wait)
                    for x in inst.ins:
                        n = getattr(x, "memref", None)
                        if n not in dram_names or n not in stores:
                            continue
                        lo, hi = _rng(x)
                        for s0, s1, ant, val, sid in stores[n]:
                            if s1 <= lo or s0 >= hi:
                                continue
                            if not any(w.ant_name == ant
                                       and w.wait_value >= val
                                       for w in new_waits):
                                new_waits.append(_br.SyncWait(
                                    sync_type="semaphore", id=sid,
                                    ant_name=ant,
                                    wait_mode="sem-ge-imm",
                                    wait_value=val, wait_reg=None))
                    if len(new_waits) != len(si.on_wait):
                        si.on_wait = new_waits
                        inst.sync_info = si
                for u in ups:
                    if (u.ant_name or "").startswith(("DMASW", "DMAHW")):
                        sem_cum[u.ant_name] = sem_cum.get(u.ant_name, 0) \
                            + u.update_value
                if "DMACopy" in type(inst).__name__:
                    for o in inst.outs:
                        n = getattr(o, "memref", None)
                        if n in dram_names:
                            lo, hi = _rng(o)
                            for u in ups:
                                if (u.ant_name or "").startswith(
                                        ("DMASW", "DMAHW")):
                                    stores.setdefault(n, []).append(
                                        (lo, hi, u.ant_name,
                                         sem_cum[u.ant_name], u.id))


def _prune_tail_drain(nc):
    """The kernel-tail drain waits on every engine and every DMA queue used,
    which overflows the drain ISA struct's wait slots. Every input DMA here
    is awaited by a compute consumer and every compute chain feeds the
    output stores, so the only waits the drain truly needs are the compute
    engines and the queue sems of DMAs that write ExternalOutput tensors."""
    out_names = set()
    for fn in nc.m.functions:
        for alloc in fn.allocations:
            if getattr(alloc, "kind", None) == "ExternalOutput":
                for ml in alloc.memorylocations:
                    out_names.add(ml.name)
    keep_dma_sems = set()
    for fn in nc.m.functions:
        for blk in fn.blocks:
            for inst in blk.instructions:
                if "DMACopy" not in type(inst).__name__:
                    continue
                if any(getattr(o, "memref", None) in out_names
                       for o in inst.outs):
                    si = inst.sync_info
                    if si:
                        for u in si.on_update:
                            keep_dma_sems.add(u.ant_name)
    for fn in nc.m.functions:
        for blk in fn.blocks:
            for inst in blk.instructions:
                if "Drain" not in type(inst).__name__:
                    continue
                si = inst.sync_info
                if si is None or len(si.on_wait) <= 2:
                    continue
                kept = [w for w in si.on_wait
                        if w.ant_name in keep_dma_sems]
                if len(kept) != len(si.on_wait):
                    si.on_wait = kept
                    inst.sync_info = si


def build_program():
    nc = bass.Bass()

    xb = nc.dram_tensor("xb", [C, N], F32R, kind="ExternalInput")
    wqk = nc.dram_tensor("wqk", [C, 2 * C], F32R, kind="ExternalInput")
    wv = nc.dram_tensor("wv", [C, H * (D + 1)], F32R, kind="ExternalInput")
    wo = nc.dram_tensor("wo", [C, C], F32R, kind="ExternalInput")
    bo = nc.dram_tensor("bo", [C, 1], F32, kind="ExternalInput")
    nw = nc.dram_tensor("nw", [C, 1], F32, kind="ExternalInput")
    nb = nc.dram_tensor("nb", [C, 1], F32, kind="ExternalInput")
    out = nc.dram_tensor("out", [C, S], F32, kind="ExternalOutput")

    with tile.TileContext(nc) as tc:
        with (
            tc.tile_pool(name="const", bufs=1) as cpool,
            tc.tile_pool(name="wst", bufs=1) as wst,
            tc.tile_pool(name="xpool", bufs=1) as xpool,
            tc.tile_pool(name="kq", bufs=1) as kqpool,
            tc.tile_pool(name="vt", bufs=1) as vtpool,
            tc.tile_pool(name="small", bufs=1) as small,
            tc.tile_pool(name="dram", bufs=1, space="DRAM") as dpool,
            tc.tile_pool(name="psum", bufs=2, space="PSUM") as psum,
        ):
            # ---- group masks for the stat matmuls, via DVE memsets ----
            gmask = [cpool.tile([128, G], F32, name=f"gmask{i}", tag=f"gmask{i}")
                     for i in range(CT)]
            for ct in range(CT):
                for quad in range(4):
                    rows = slice(32 * quad, 32 * (quad + 1))
                    other = slice(4 - 4 * ct, 8 - 4 * ct)
                    nc.vector.memset(gmask[ct][rows, other], 0.0)
                    for g in range(4):
                        col = slice(4 * ct + g, 4 * ct + g + 1)
                        nc.vector.memset(gmask[ct][rows, col],
                                         1.0 if g == quad else 0.0)
            zerob = cpool.tile([128, 1], F32, name="zerob", tag="zerob")
            nc.vector.memset(zerob[:], 0.0)
            epsb = cpool.tile([G, 1], F32, name="epsb", tag="epsb")
            nc.vector.memset(epsb[:], EPS)
            ones_f = wst.tile([128, G], F32, name="ones_f", tag="ones_f")
            nc.vector.memset(ones_f[:], 1.0)
            ones8r = cpool.tile([128, G], BF16, name="ones8r", tag="ones8r")
            nc.vector.tensor_copy(ones8r[:], ones_f[:])
            # warm the natural_log_exp_and_others ACT table set with a dummy
            # op whose only dependency is DVE, so no later activation pays
            # the table-load pseudo-instruction's extra sync wait
            actwarm = cpool.tile([128, 1], F32, name="actwarm", tag="actwarm")
            nc.scalar.activation(actwarm[:], zerob[:], AF.Ln, bias=1.0,
                                 scale=1.0)

            # ---- weights: DMA into staging, DVE-copy to compute tiles so
            # downstream PE/ACT instructions never wait on DMA queues ----
            wqk_s = [wst.tile([128, 2 * C], F32R, name=f"wqks{i}", tag=f"wqks{i}")
                     for i in range(CT)]
            wv_s = [wst.tile([128, H * (D + 1)], F32R, name=f"wvs{i}", tag=f"wvs{i}")
                    for i in range(CT)]
            wo_s = [wst.tile([128, C], F32R, name=f"wos{i}", tag=f"wos{i}")
                    for i in range(CT)]
            nwnb_s = [wst.tile([128, 3], F32, name=f"nns{i}", tag=f"nns{i}")
                      for i in range(CT)]
            wqk_t = [cpool.tile([128, 2 * C], F32R, name=f"wqk{i}", tag=f"wqk{i}")
                     for i in range(CT)]
            wv_t = [cpool.tile([128, H * (D + 1)], F32R, name=f"wv{i}", tag=f"wv{i}")
                    for i in range(CT)]
            wo_t = [cpool.tile([128, C], F32R, name=f"wo{i}", tag=f"wo{i}")
                    for i in range(CT)]
            nw_t = [cpool.tile([128, 1], F32, name=f"nw{i}", tag=f"nw{i}")
                    for i in range(CT)]
            nb_t = [cpool.tile([128, 1], F32, name=f"nb{i}", tag=f"nb{i}")
                    for i in range(CT)]
            bo_t = [cpool.tile([128, 1], F32, name=f"bo{i}", tag=f"bo{i}")
                    for i in range(CT)]
            x_t = [xpool.tile([128, N], F32R, name=f"x{i}", tag=f"x{i}")
                   for i in range(CT)]
            for ct in range(CT):
                nc.sync.dma_start(x_t[ct][:], xb[128 * ct:128 * (ct + 1), :])
            for ct in range(CT):
                sl = slice(128 * ct, 128 * (ct + 1))
                nc.sync.dma_start(wqk_s[ct][:], wqk[sl, :])
                nc.sync.dma_start(wv_s[ct][:], wv[sl, :])
                nc.sync.dma_start(wo_s[ct][:], wo[sl, :])
                nc.sync.dma_start(nwnb_s[ct][:, 0:1], nw[sl, :])
                nc.sync.dma_start(nwnb_s[ct][:, 1:2], nb[sl, :])
                nc.sync.dma_start(nwnb_s[ct][:, 2:3], bo[sl, :])
            for ct in range(CT):
                nc.vector.tensor_copy(wqk_t[ct][:], wqk_s[ct][:])
                nc.vector.tensor_copy(wv_t[ct][:], wv_s[ct][:])
                nc.vector.tensor_copy(wo_t[ct][:], wo_s[ct][:])
                nc.vector.tensor_copy(nw_t[ct][:], nwnb_s[ct][:, 0:1])
                nc.vector.tensor_copy(nb_t[ct][:], nwnb_s[ct][:, 1:2])
                nc.vector.tensor_copy(bo_t[ct][:], nwnb_s[ct][:, 2:3])

            # ---- load x ----
            xres = [xpool.tile([128, S], F32, name=f"xres{i}", tag=f"xres{i}")
                    for i in range(CT)]
            for ct in range(CT):
                nc.vector.tensor_copy(xres[ct][:], x_t[ct][:, 0:S])

            attn_d = [kqpool.tile([128, S], F32R, name=f"at{i}", tag=f"at{i}")
                      for i in range(CT)]
            k_t = [kqpool.tile([128, N], BF16, name=f"k{i}", tag=f"k{i}")
                   for i in range(CT)]
            q_t = [kqpool.tile([128, S], BF16, name=f"q{i}", tag=f"q{i}")
                   for i in range(CT)]
            vt_all = vtpool.tile([128, TT, H, D + 1], BF16, name="vt", tag="vt")
            # persistent, manually double-buffered exp output (a rotating
            # pool slot here would add a second sync wait on the ACT op)
            e_all = vtpool.tile([128, 2, 2, 512], BF16, name="e_all",
                                tag="e_all")
            # softmax denominators, collected on one partition per group
            # slot (2h + sblk), reciprocal'd in bulk at the end
            den_all = vtpool.tile([1, 2 * H * 512], F32, name="den_all",
                                  tag="den_all")

            # ---- groupnorm stats ----
            stats = [small.tile([128, 2], F32, name=f"st{i}", tag=f"st{i}")
                     for i in range(CT)]
            xsq = [wst.tile([128, N], F32, name=f"xsq{i}", tag=f"xsq{i}")
                   for i in range(CT)]
            if True:
                gstat_p = psum.tile([G, 2], F32, name="gstat", tag="pv")
                for ct in range(CT):
                    nc.vector.reduce_sum(stats[ct][:, 0:1], x_t[ct][:],
                                         axis=mybir.AxisListType.X)
                    # x*x into a scratch tile, then reduce
                    nc.vector.tensor_mul(xsq[ct][:], x_t[ct][:], x_t[ct][:])
                    nc.vector.reduce_sum(stats[ct][:, 1:2], xsq[ct][:],
                                         axis=mybir.AxisListType.X)
                for ct in range(CT):
                    nc.tensor.matmul(gstat_p[:], gmask[ct][:], stats[ct][:],
                                     start=(ct == 0), stop=(ct == CT - 1))
                MTOT = float(32 * N)
                ms = small.tile([G, 2], F32, name="ms", tag="ms")   # [mean, rstd]
                ex2 = small.tile([G, 1], F32, name="ex2", tag="ex2")
                nc.vector.tensor_scalar_mul(ms[:, 0:1], gstat_p[:, 0:1], 1.0 / MTOT)
                nc.vector.tensor_scalar_mul(ex2[:], gstat_p[:, 1:2], 1.0 / MTOT)
            m2 = small.tile([G, 1], F32, name="m2", tag="m2")
            nc.vector.tensor_mul(m2[:], ms[:, 0:1], ms[:, 0:1])
            var = small.tile([G, 1], F32, name="var", tag="var")
            nc.vector.tensor_sub(var[:], ex2[:], m2[:])
            sd = small.tile([G, 1], F32, name="sd", tag="sd")
            nc.scalar.activation(sd[:], var[:], AF.Ln, bias=epsb[:], scale=1.0)
            rstd8 = small.tile([G, 1], F32, name="rstd8", tag="rstd8")
            nc.scalar.activation(rstd8[:], sd[:], AF.Exp, bias=zerob[0:G, :],
                                 scale=-0.5)
            nc.vector.tensor_copy(ms[:, 1:2], rstd8[:])

            # broadcast [8,2] group stats to [128,2] per c-tile via a DRAM
            # round-trip with a replicating read pattern
            ms_d = dpool.tile([G, 2], F32, name="msd", tag="msd")
            nc.gpsimd.dma_start(ms_d[:], ms[:])
            mr = [small.tile([128, 2], F32, name=f"mr{i}", tag=f"mr{i}")
                  for i in range(CT)]
            for ct in range(CT):
                src = bass.AP(tensor=ms_d.tensor,
                              offset=ms_d.offset + 8 * ct,
                              ap=[[2, 4], [0, 32], [1, 2]])
                nc.gpsimd.dma_start(mr[ct][:], src)

            # per-channel affine: xn = x * weff + beff (in place on x_t)
            for ct in range(CT):
                weff = small.tile([128, 1], F32, name=f"weff{ct}", tag=f"weff{ct}")
                beff = small.tile([128, 1], F32, name=f"beff{ct}", tag=f"beff{ct}")
                nc.vector.tensor_mul(weff[:], mr[ct][:, 1:2], nw_t[ct][:])
                nc.vector.tensor_mul(beff[:], mr[ct][:, 0:1], weff[:])
                nc.vector.tensor_sub(beff[:], nb_t[ct][:], beff[:])
                nc.vector.tensor_scalar(out=x_t[ct][:], in0=x_t[ct][:],
                                        scalar1=weff[:], scalar2=beff[:],
                                        op0=ALU.mult, op1=ALU.add)
            xn = x_t  # normalized in place

            # ---- projections: V^T first (so its DVE writes are observed by
            # the PE before any attention matmul), then Q, then K ----
            if True:
                # V^T: [token, head*(D+1)] directly from xn as stationary
                for ti in range(TT):
                    vp = psum.tile([128, H * (D + 1)], F32, name="vp", tag="sc")
                    for ci in range(CT):
                        nc.tensor.matmul(
                            vp[:],
                            xn[ci][:, 128 * ti:128 * (ti + 1)],
                            wv_t[ci][:],
                            start=(ci == 0), stop=(ci == CT - 1))
                    nc.vector.tensor_copy(vt_all[:, ti, :, 0:D],
                                          vp[:].rearrange(
                                              "p (h e) -> p h e",
                                              e=D + 1)[:, :, 0:D])
                    # each head's extra column holds ones (the PV
                    # denominator row); strided copy from the ones tile
                    nc.vector.tensor_copy(vt_all[:, ti, :, D], ones8r[:])
                # Q: out channels 0:256 of wqk cols; only S token columns
                for sblk in range(SB):
                    for m in range(CT):
                        qp = psum.tile([128, 512], F32, name="qp", tag="sc")
                        for ci in range(CT):
                            nc.tensor.matmul(
                                qp[:],
                                wqk_t[ci][:, 128 * m:128 * (m + 1)],
                                xn[ci][:, 512 * sblk:512 * (sblk + 1)],
                                start=(ci == 0), stop=(ci == CT - 1))
                        nc.vector.tensor_copy(
                            q_t[m][:, 512 * sblk:512 * (sblk + 1)], qp[:])
                # K: out channels 256:512 of wqk cols; all N tokens
                for nblk in range(N // 512):
                    for m in range(CT):
                        kp = psum.tile([128, 512], F32, name="kp", tag="sc")
                        for ci in range(CT):
                            nc.tensor.matmul(
                                kp[:],
                                wqk_t[ci][:, C + 128 * m:C + 128 * (m + 1)],
                                xn[ci][:, 512 * nblk:512 * (nblk + 1)],
                                start=(ci == 0), stop=(ci == CT - 1))
                        nc.vector.tensor_copy(
                            k_t[m][:, 512 * nblk:512 * (nblk + 1)], kp[:])

            # ---- attention: flat software-pipelined stream ----
            # groups: (htile, row-band pair, sblk); head pair (hA, hB) sits
            # at k_t/q_t partition bands (hpA, hpA+64) so the two QK
            # matmuls row-tile into disjoint PE row groups, and the two PV
            # matmuls col-tile into output partitions 0:33 / 64:97.
            attn = attn_d
            groups = []
            for htile in range(CT):
                for lo in range(2):
                    hA = 4 * htile + lo          # row band 32*lo
                    hB = 4 * htile + lo + 2      # row band 32*lo + 64
                    for sblk in range(SB):
                        groups.append((htile, 32 * lo, hA, hB, sblk))

            NG = len(groups)                     # 16 groups x 32 t-tiles
            steps = [(g, ti) for g in range(NG) for ti in range(TT)]
            pv_tiles = [None] * NG
            retire = [None] * NG                 # last DVE copyout per group
            pend = []                            # PV args pending emission

            def emit_pv(g, ti, e):
                htile, hp, hA, hB, sblk = groups[g]
                pv = pv_tiles[g]
                nc.tensor.matmul(pv[0:D + 1, :], vt_all[:, ti, hA, :],
                                 e[:, 0, :], start=(ti == 0),
                                 stop=(ti == TT - 1), tile_position=(0, 0))
                nc.tensor.matmul(pv[64:64 + D + 1, :], vt_all[:, ti, hB, :],
                                 e[:, 1, :], start=(ti == 0),
                                 stop=(ti == TT - 1), tile_position=(0, 64))
                if ti == TT - 1:
                    # group done: unnormalized O rows to SBUF, denominator
                    # rows to the per-(head, sblk) collector slots
                    ssl = slice(512 * sblk, 512 * (sblk + 1))
                    nc.vector.tensor_copy(attn[htile][hp:hp + 32, ssl],
                                          pv[0:D, :])
                    nc.vector.tensor_copy(attn[htile][hp + 64:hp + 96, ssl],
                                          pv[64:64 + D, :])
                    nc.vector.tensor_copy(
                        den_all[:, 512 * (2 * hA + sblk):
                                512 * (2 * hA + sblk) + 512],
                        pv[D:D + 1, :])
                    retire[g] = nc.vector.tensor_copy(
                        den_all[:, 512 * (2 * hB + sblk):
                                512 * (2 * hB + sblk) + 512],
                        pv[64 + D:64 + D + 1, :])

            for k, (g, ti) in enumerate(steps):
                htile, hp, hA, hB, sblk = groups[g]
                if ti == 0:
                    pv_tiles[g] = psum.tile([128, 512], F32,
                                            name=f"pv{g}", tag="pv")
                ssl = slice(512 * sblk, 512 * (sblk + 1))
                tsl = slice(128 * ti, 128 * (ti + 1))
                st = psum.tile([128, 2, 512], F32, name="st", tag="sc")
                nc.tensor.matmul(st[:, 0, :], k_t[htile][hp:hp + 32, tsl],
                                 q_t[htile][hp:hp + 32, ssl],
                                 start=True, stop=True,
                                 tile_position=(hp, 0))
                mm2 = nc.tensor.matmul(st[:, 1, :],
                                       k_t[htile][hp + 64:hp + 96, tsl],
                                       q_t[htile][hp + 64:hp + 96, ssl],
                                       start=True, stop=True,
                                       tile_position=(hp + 64, 0))
                if ti == 0 and g >= 2:
                    # pin the DVE->PE edge that retires the PV accumulator
                    # bank two groups back on the second QK matmul (whose
                    # own WAR is covered by the first's); the PV start=True
                    # then needs no DVE wait of its own
                    add_dep_helper(mm2.ins, retire[g - 2].ins,
                                   reason="retire pv bank")
                e = e_all[:, k % 2]
                nc.scalar.activation(e[:, :, :], st[:, :, :], AF.Exp,
                                     bias=zerob[:], scale=SCALE)
                # PV for the PREVIOUS step, so the PE never stalls on exp
                if pend:
                    emit_pv(*pend.pop())
                pend.append((g, ti, e))
            emit_pv(*pend.pop())

            # ---- bulk softmax normalization: 1/denom via exp(-ln(x)) on
            # ACT (same table set as the softmax exp), gathered to 128
            # partitions and broadcast to [128, S] per c-tile via DRAM
            # round-trips with replicating read patterns ----
            den_d = dpool.tile([2 * H * 512], F32, name="dend", tag="dend")
            nc.gpsimd.dma_start(den_d[:], den_all[:])
            den_g = small.tile([128, 64], F32, name="deng", tag="deng")
            gsrc = bass.AP(tensor=den_d.tensor, offset=den_d.offset,
                           ap=[[64, 128], [1, 64]])
            nc.gpsimd.dma_start(den_g[:], gsrc)
            lnd = small.tile([128, 64], F32, name="lnd", tag="lnd")
            nc.scalar.activation(lnd[:], den_g[:], AF.Ln, bias=zerob[:],
                                 scale=1.0)
            rden = small.tile([128, 64], F32, name="rden", tag="rden")
            nc.scalar.activation(rden[:], lnd[:], AF.Exp, bias=zerob[:],
                                 scale=-1.0)
            rcp_d = dpool.tile([2 * H * 512], F32, name="rcpd", tag="rcpd")
            gdst = bass.AP(tensor=rcp_d.tensor, offset=rcp_d.offset,
                           ap=[[64, 128], [1, 64]])
            nc.gpsimd.dma_start(gdst, rden[:])
            for ci in range(CT):
                rbc = small.tile([128, S], F32, name=f"rbc{ci}", tag=f"rbc{ci}")
                bsrc = bass.AP(tensor=rcp_d.tensor,
                               offset=rcp_d.offset + 4096 * ci,
                               ap=[[1024, 4], [0, 32], [512, 2], [1, 512]])
                nc.gpsimd.dma_start(rbc[:], bsrc)
                nc.vector.tensor_mul(attn[ci][:], attn[ci][:], rbc[:])

            # ---- output projection + bias + residual ----
            out_all = small.tile([128, CT * S], F32, name="out_all",
                                 tag="out_all")
            out_t = [out_all[:, S * i:S * (i + 1)] for i in range(CT)]
            if True:
                for sblk in range(SB):
                    ssl = slice(512 * sblk, 512 * (sblk + 1))
                    for m in range(CT):
                        op = psum.tile([128, 512], F32, name="op", tag="pv")
                        for ci in range(CT):
                            nc.tensor.matmul(
                                op[:],
                                wo_t[ci][:, 128 * m:128 * (m + 1)],
                                attn[ci][:, ssl],
                                start=(ci == 0), stop=(ci == CT - 1))
                        nc.vector.scalar_tensor_tensor(
                            out=out_t[m][:, ssl], in0=op[:], scalar=bo_t[m][:],
                            in1=xres[m][:, ssl], op0=ALU.add, op1=ALU.add)

            out_r = out.rearrange("(m p) s -> p m s", p=128)
            nc.gpsimd.dma_start(
                out_r, out_all[:].rearrange("p (m s) -> p m s", m=CT))

    _strip_same_engine_waits(nc)
    _fix_dram_readback_waits(nc)
    _prune_tail_drain(nc)
    return nc


def shard_inputs(x, norm_w, norm_b, w_qkv, w_out, b_out):
    """Build the 8 per-core input maps from the full problem inputs."""
    x = np.asarray(x, np.float32).reshape(B, C, N)
    w_qkv = np.asarray(w_qkv, np.float32)
    w_out = np.asarray(w_out, np.float32)
    wqkT = np.ascontiguousarray(w_qkv[:2 * C].T)          # [C, 512]
    wv = np.zeros((C, H * (D + 1)), np.float32)           # [C, 264]
    for h in range(H):
        wv[:, (D + 1) * h:(D + 1) * h + D] = w_qkv[2 * C + D * h:2 * C + D * (h + 1)].T
    woT = np.ascontiguousarray(w_out.T)
    nwc = np.ascontiguousarray(np.asarray(norm_w, np.float32).reshape(C, 1))
    nbc = np.ascontiguousarray(np.asarray(norm_b, np.float32).reshape(C, 1))
    boc = np.ascontiguousarray(np.asarray(b_out, np.float32).reshape(C, 1))
    in_maps = []
    for i in range(NCORES):
        b, j = i // 4, i % 4
        xrot = np.ascontiguousarray(np.roll(x[b], -S * j, axis=1))
        in_maps.append({"xb": xrot, "wqk": wqkT, "wv": wv, "wo": woT,
                        "bo": boc, "nw": nwc, "nb": nbc})
    return in_maps


def kernel_with_info(x, norm_w, norm_b, w_qkv, w_out, b_out):
    in_maps = shard_inputs(x, norm_w, norm_b, w_qkv, w_out, b_out)
    nc = build_program()
    trace = bool(os.environ.get("BASS_TRACE"))
    res = run_bass_kernel_spmd(nc, in_maps, core_ids=list(range(NCORES)),
                               trace=trace)
    full = np.empty((B, C, N), np.float32)
    for i in range(NCORES):
        b, j = i // 4, i % 4
        full[b][:, S * j:S * (j + 1)] = res.results[i]["out"]
    return full.reshape(B, C, 16, 16, 16), res


def kernel(x, norm_w, norm_b, w_qkv, w_out, b_out):
    out, _ = kernel_with_info(x, norm_w, norm_b, w_qkv, w_out, b_out)
    return out
